# revision 1
# baseline (speedup 1.0000x reference)
"""Bond-message embedding kernel for TRN2 (8 NeuronCores, SPMD).

Computes out[e, :] = concat(V[src[e]], E[e]) @ W.T + b for 800k edges.

Sharding: edges split evenly across the 8 cores (data-parallel over the
edge dim); V, W, b replicated; no cross-core comm. Within each core the
edges are reordered into a low group (src < 32768) and a high group
(src >= 32768) so the custom dma_gather ucode (int16 indices) can gather
V rows with a per-group base offset; the host undoes the reordering when
it reassembles the full output.

Per-core device pipeline (per 1024-edge chunk):
  1. Load wrapped int16 gather indices [128, 64].
  2. dma_gather pulls 1024 V rows (512B each) into SBUF edge-major
     (64+1 descriptors per SDMA lane — fits the SWDGE ring).
  3. Load the E chunk edge-major (2KB contiguous per partition).
  4. Per 128-edge block: PE-transpose V/E blocks to feature-major
     (batched 4 blocks per PSUM bank), evacuate PSUM->SBUF on the
     scalar engine (rounding to float32r), two accumulating float32r
     matmuls (full-rate fp32) into PSUM [128, 512] (2 blocks/bank),
     DVE adds the partition-replicated bias while evacuating PSUM,
     and one DMA per chunk stores the 1024 output rows.
"""

import numpy as np

import concourse.bacc as bacc
import concourse.mybir as mybir
import concourse.tile as tile
from concourse.bass_utils import run_bass_kernel_spmd
from concourse.masks import make_identity

F32 = mybir.dt.float32
F32R = mybir.dt.float32r
I16 = mybir.dt.int16

N_CORES = 8
N_NODES = 50000
ATOM = 128
BOND = 64
MSG = 256
N_EDGES = 800000
SPLIT = 32768          # int16-safe index boundary

PER_CORE_RAW = N_EDGES // N_CORES   # 100000
K_BLK = 8                           # 128-edge blocks per chunk
CHUNK = 128 * K_BLK                 # 1024 edges per chunk
LOW_CHUNKS = 65                     # 66560 low-group slots (expect ~65536)
HIGH_CHUNKS = 35                    # 35840 high-group slots (expect ~34464)
N_CHUNKS = LOW_CHUNKS + HIGH_CHUNKS
LOW_SLOTS = LOW_CHUNKS * CHUNK
HIGH_SLOTS = HIGH_CHUNKS * CHUNK
PER_CORE = N_CHUNKS * CHUNK         # 104448 device slots per core

P = 128
IDX_COLS = CHUNK // 16              # 64

G_TR = 4                            # transposes batched per PSUM bank


def _emit_pipeline(nc, tc, n_chunks, low_chunks, k_blk, n_nodes, split,
                   handles, reps=1, n_queues=1):
    v_h, e_h, idx_h, wt_h, b_h, out_h = handles
    chunk = P * k_blk
    idx_cols = chunk // 16

    with (
        tc.tile_pool(name="const", bufs=1) as const,
        tc.tile_pool(name="chunkio", bufs=3) as chunkio,
        tc.tile_pool(name="work", bufs=4) as work,
        tc.tile_pool(name="psum", bufs=2, space="PSUM") as psum,
    ):
        # --- constants -----------------------------------------------------
        ident = const.tile([P, P], F32)
        make_identity(nc, ident[:])

        wt_stage = const.tile([P, 2 * MSG], F32)
        nc.sync.dma_start(out=wt_stage[:, 0:MSG], in_=wt_h[0:ATOM, :])
        nc.sync.dma_start(
            out=wt_stage[0:BOND, MSG:2 * MSG], in_=wt_h[ATOM:ATOM + BOND, :]
        )
        w1t = const.tile([ATOM, MSG], F32R)   # W.T rows 0:128 (atom part)
        nc.vector.tensor_copy(out=w1t[:], in_=wt_stage[:, 0:MSG])
        w2t = const.tile([BOND, MSG], F32R)   # W.T rows 128:192 (bond part)
        nc.vector.tensor_copy(out=w2t[:], in_=wt_stage[0:BOND, MSG:2 * MSG])

        b_row = const.tile([1, MSG], F32)
        nc.sync.dma_start(out=b_row[:], in_=b_h[:, :])
        ones_col = const.tile([1, P], F32)
        nc.vector.memset(ones_col[:], 1.0)
        # replicate bias across all 128 partitions via outer product
        bias_ps = psum.tile([P, MSG], F32, tag="biasps")
        nc.tensor.matmul(
            out=bias_ps[:], lhsT=ones_col[:], rhs=b_row[:], start=True, stop=True
        )
        # doubled bias [128, 512] so a 2-block PSUM evac adds it in one op
        bias2 = const.tile([P, 2 * MSG], F32)
        nc.vector.tensor_copy(out=bias2[:, 0:MSG], in_=bias_ps[:])
        nc.vector.tensor_copy(out=bias2[:, MSG:2 * MSG], in_=bias_ps[:])

        # --- main loop -----------------------------------------------------
        def chunk_body():
            for c in range(n_chunks):
                c0 = c * chunk
                c1 = c0 + chunk
                v_base = (
                    v_h[0:split, :] if c < low_chunks else v_h[split:n_nodes, :]
                )

                idx_t = chunkio.tile([P, idx_cols], I16, tag="idx")
                nc.sync.dma_start(
                    out=idx_t[:], in_=idx_h[c * P:(c + 1) * P, :]
                )
                g_t = chunkio.tile([P, k_blk * ATOM], F32, tag="gather")
                nc.gpsimd.dma_gather(
                    out_ap=g_t[:].rearrange("p (k f) -> p k f", f=ATOM),
                    in_ap=v_base,
                    idxs_ap=idx_t[:, :],
                    num_idxs=chunk,
                    num_idxs_reg=chunk,
                    elem_size=ATOM,
                    single_packet=False,
                    queue_num=c % n_queues,
                )
                e_t = chunkio.tile([P, k_blk * BOND], F32, tag="ebond")
                nc.sync.dma_start(
                    out=e_t[:],
                    in_=e_h[c0:c1, :].rearrange("(p k) f -> p (k f)", p=P),
                )

                out_view = out_h[c0:c1, :].rearrange("(p k) m -> p k m", p=P)
                o_grp = work.tile([P, k_blk * MSG], F32, tag="ogrp")

                for vg in range(0, k_blk, G_TR):
                    nv = min(G_TR, k_blk - vg)
                    # batched feature-major transposes (one PSUM bank each)
                    vt_ps = psum.tile([P, G_TR * P], F32, tag="vtps")
                    et_ps = psum.tile([BOND, G_TR * P], F32, tag="etps")
                    for t in range(nv):
                        j = vg + t
                        nc.tensor.transpose(
                            out=vt_ps[:, t * P:(t + 1) * P],
                            in_=g_t[:, j * ATOM:(j + 1) * ATOM],
                            identity=ident[:],
                        )
                        nc.tensor.transpose(
                            out=et_ps[:, t * P:(t + 1) * P],
                            in_=e_t[:, j * BOND:(j + 1) * BOND],
                            identity=ident[:],
                        )
                    vt_sb = work.tile([P, G_TR * P], F32R, tag="vtsb")
                    nc.scalar.copy(out=vt_sb[:, 0:nv * P], in_=vt_ps[:, 0:nv * P])
                    et_sb = work.tile([BOND, G_TR * P], F32R, tag="etsb")
                    nc.scalar.copy(out=et_sb[:, 0:nv * P], in_=et_ps[:, 0:nv * P])

                    for t2 in range(0, nv, 2):
                        nn = min(2, nv - t2)
                        o_ps = psum.tile([P, 2 * MSG], F32, tag="ops")
                        for u in range(nn):
                            t = t2 + u
                            nc.tensor.matmul(
                                out=o_ps[:, u * MSG:(u + 1) * MSG],
                                lhsT=vt_sb[:, t * P:(t + 1) * P],
                                rhs=w1t[:],
                                start=True,
                                stop=False,
                            )
                            nc.tensor.matmul(
                                out=o_ps[:, u * MSG:(u + 1) * MSG],
                                lhsT=et_sb[:, t * P:(t + 1) * P],
                                rhs=w2t[:],
                                start=False,
                                stop=True,
                            )
                        # bias add fused with 2-block PSUM->SBUF evacuation
                        d0 = (vg + t2) * MSG
                        nc.vector.tensor_tensor(
                            out=o_grp[:, d0:d0 + nn * MSG],
                            in0=o_ps[:, 0:nn * MSG],
                            in1=bias2[:, 0:nn * MSG],
                            op=mybir.AluOpType.add,
                        )

                nc.sync.dma_start(out=out_view[:, :, :], in_=o_grp[:, :])

        if reps == 1:
            chunk_body()
        else:
            with tc.For_i(0, reps, 1):
                chunk_body()


def build_nc(n_chunks=N_CHUNKS, low_chunks=LOW_CHUNKS, k_blk=K_BLK,
             n_nodes=N_NODES, split=SPLIT, reps=1, n_queues=1):
    chunk = P * k_blk
    per_core = n_chunks * chunk
    idx_cols = chunk // 16

    nc = bacc.Bacc(num_swdge_queues=n_queues)
    handles = (
        nc.declare_dram_parameter("V", [n_nodes, ATOM], F32, isOutput=False),
        nc.declare_dram_parameter("E", [per_core, BOND], F32, isOutput=False),
        nc.declare_dram_parameter(
            "idx16", [n_chunks * P, idx_cols], I16, isOutput=False
        ),
        nc.declare_dram_parameter("Wt", [ATOM + BOND, MSG], F32, isOutput=False),
        nc.declare_dram_parameter("b", [1, MSG], F32, isOutput=False),
        nc.declare_dram_parameter("out", [per_core, MSG], F32, isOutput=True),
    )
    with tile.TileContext(nc) as tc:
        _emit_pipeline(nc, tc, n_chunks, low_chunks, k_blk, n_nodes, split,
                       handles, reps=reps, n_queues=n_queues)
    return nc


def build_nc_null():
    """Null kernel with identical I/O signature — for RPC/transfer calibration."""
    nc = bacc.Bacc()
    nc.declare_dram_parameter("V", [N_NODES, ATOM], F32, isOutput=False)
    nc.declare_dram_parameter("E", [PER_CORE, BOND], F32, isOutput=False)
    nc.declare_dram_parameter("idx16", [N_CHUNKS * P, IDX_COLS], I16, isOutput=False)
    wt_h = nc.declare_dram_parameter("Wt", [ATOM + BOND, MSG], F32, isOutput=False)
    nc.declare_dram_parameter("b", [1, MSG], F32, isOutput=False)
    out_h = nc.declare_dram_parameter("out", [PER_CORE, MSG], F32, isOutput=True)
    with tile.TileContext(nc) as tc:
        with tc.tile_pool(name="p", bufs=1) as pool:
            t = pool.tile([P, MSG], F32)
            nc.sync.dma_start(out=t[:], in_=wt_h[0:P, :])
            nc.sync.dma_start(out=out_h[0:P, :], in_=t[:])
    return nc


_NC_CACHE = {}


def _get_nc(key, **kw):
    if key not in _NC_CACHE:
        nc = (build_nc_null if key == "null" else build_nc)(**kw)
        nc.finalize()  # run Bacc passes (reg alloc, matmul wait legalization)
        _NC_CACHE[key] = nc
    return _NC_CACHE[key]


def wrap_idx16(idx_dev_chunk, k_blk=K_BLK):
    """Per-chunk gather-index packing for dma_gather.

    Device slot s = p*k_blk + j lands at gather position i = j*128 + p
    (ucode writes position i to [partition i%128, block i//128]). The
    wrapped layout stores position i at [partition i%16, col i//16],
    replicated across the 8 16-partition bands.
    """
    chunk = P * k_blk
    gather_order = idx_dev_chunk.reshape(P, k_blk).T.ravel()        # [chunk]
    w16 = gather_order.reshape(chunk // 16, 16).T                   # [16, cols]
    return np.tile(w16, (8, 1))                                     # [128, cols]


def _make_in_maps(V, E, edge_index, W, b):
    V = np.ascontiguousarray(np.asarray(V, dtype=np.float32))
    E = np.asarray(E, dtype=np.float32)
    W = np.asarray(W, dtype=np.float32)
    b = np.asarray(b, dtype=np.float32)

    src = np.asarray(edge_index[0]).astype(np.int32)
    wt = np.ascontiguousarray(W.T)            # [192, 256]
    b_row = np.ascontiguousarray(b[None, :])  # [1, 256]

    in_maps = []
    placements = []
    for i in range(N_CORES):
        lo = i * PER_CORE_RAW
        src_i = src[lo:lo + PER_CORE_RAW]
        e_i = E[lo:lo + PER_CORE_RAW]

        low_pos = np.flatnonzero(src_i < SPLIT)
        high_pos = np.flatnonzero(src_i >= SPLIT)
        n_low, n_high = len(low_pos), len(high_pos)
        assert n_low <= LOW_SLOTS and n_high <= HIGH_SLOTS, (n_low, n_high)

        e_dev = np.zeros((PER_CORE, BOND), np.float32)
        e_dev[:n_low] = e_i[low_pos]
        e_dev[LOW_SLOTS:LOW_SLOTS + n_high] = e_i[high_pos]

        idx_dev = np.zeros(PER_CORE, np.int32)
        idx_dev[:n_low] = src_i[low_pos]
        idx_dev[LOW_SLOTS:LOW_SLOTS + n_high] = src_i[high_pos] - SPLIT

        # vectorized wrap over all chunks at once:
        # [n_chunks, P, k_blk] -> gather order [n_chunks, k_blk, P]
        a = idx_dev.reshape(N_CHUNKS, P, K_BLK).transpose(0, 2, 1)
        a = a.reshape(N_CHUNKS, CHUNK // 16, 16).transpose(0, 2, 1)  # [nc,16,cols]
        idx16 = np.ascontiguousarray(
            np.tile(a, (1, 8, 1)).reshape(N_CHUNKS * P, IDX_COLS).astype(np.int16)
        )

        in_maps.append(
            {
                "V": V,
                "E": np.ascontiguousarray(e_dev),
                "idx16": idx16,
                "Wt": wt,
                "b": b_row,
            }
        )
        placements.append((low_pos, high_pos))
    return in_maps, placements


def kernel(V, E, edge_index, W, b):
    in_maps, placements = _make_in_maps(V, E, edge_index, W, b)
    nc = _get_nc("full")
    res = run_bass_kernel_spmd(nc, in_maps, core_ids=list(range(N_CORES)))
    out = np.empty((N_EDGES, MSG), np.float32)
    for i, (low_pos, high_pos) in enumerate(placements):
        dev = res.results[i]["out"]
        blk = out[i * PER_CORE_RAW:(i + 1) * PER_CORE_RAW]
        blk[low_pos] = dev[:len(low_pos)]
        blk[high_pos] = dev[LOW_SLOTS:LOW_SLOTS + len(high_pos)]
    return out


def kernel_null(V, E, edge_index, W, b):
    """Calibration: same transfers as kernel(), trivial device work."""
    in_maps, _ = _make_in_maps(V, E, edge_index, W, b)
    nc = _get_nc("null")
    res = run_bass_kernel_spmd(nc, in_maps, core_ids=list(range(N_CORES)))
    return res.results[0]["out"][0, 0]



# revision 2
# speedup vs baseline: 2.2789x; 2.2789x over previous
"""Bond-message embedding kernel for TRN2 (8 NeuronCores, SPMD).

Computes out[e, :] = concat(V[src[e]], E[e]) @ W.T + b for 800k edges.

Sharding: edges split evenly across the 8 cores (data-parallel over the
edge dim); V, W, b replicated; no cross-core comm.

v2 design (bf16 end-to-end, feature-major dataflow):
  * All device tensors are bf16 (rel-err budget 2e-2; bf16 keeps it ~5e-3).
    PSUM accumulates in f32.
  * dma_gather(transpose=True) writes the gathered V rows FEATURE-major
    ([128 atom-features, chunk edges]) so the matmul consumes the gather
    output directly as lhsT -- zero PE transposes.
  * E is host-pre-transposed to feature-major [65, n_slots] with a 65th
    row of ones; W2tb = [W.T rows 128:192; b] so the bias falls out of the
    accumulating matmul (exact f32 bias in PSUM, no extra vector op).
  * Edges are reordered host-side into a low group (src < 25000) and a
    high group (src >= 25000) so int16 gather indices work with a
    per-group base; 25 chunks of 2048 edges per group (2.4% padding).
  * Output slots are permuted so each partition stores 16 contiguous HBM
    rows (8KB descriptors); PSUM evacuation (f32 -> bf16 cast) alternates
    between the DVE and Activation engines.
  * The host undoes all permutations and converts bf16 -> f32.

Per-chunk device work (2048 edges):
  1 transpose-gather (2048 x 256B), 1 E load ([65, 2048] bf16), 16 pairs
  of accumulating bf16 matmuls ([128|65, 128] x [.., 256] -> PSUM), 8
  PSUM->SBUF cast-copies, 1 output store ([128, 16*256] bf16).
"""

import numpy as np
import ml_dtypes

import concourse.bacc as bacc
import concourse.mybir as mybir
import concourse.tile as tile
from concourse.bass_utils import run_bass_kernel_spmd

F32 = mybir.dt.float32
BF16 = mybir.dt.bfloat16
I16 = mybir.dt.int16
NP_BF16 = ml_dtypes.bfloat16

N_CORES = 8
N_NODES = 50000
ATOM = 128
BOND = 64
MSG = 256
N_EDGES = 800000
SPLIT = 25000          # int16-safe index boundary (both halves < 32768)

PER_CORE_RAW = N_EDGES // N_CORES   # 100000
P = 128
K_BLK = 16                          # 128-edge blocks per chunk
CHUNK = P * K_BLK                   # 2048 edges per chunk
LOW_CHUNKS = 25                     # 51200 low slots (expect ~50000)
HIGH_CHUNKS = 25                    # 51200 high slots (expect ~50000)
N_CHUNKS = LOW_CHUNKS + HIGH_CHUNKS
LOW_SLOTS = LOW_CHUNKS * CHUNK
HIGH_SLOTS = HIGH_CHUNKS * CHUNK
PER_CORE = N_CHUNKS * CHUNK         # 102400 device slots per core

IDX_COLS = CHUNK // 16              # 128 idx columns per chunk

# position q = j*128 + p within a chunk maps to out slot p*K_BLK + j
_Q = np.arange(CHUNK)
POS2SLOT = (_Q % P) * K_BLK + (_Q // P)          # [CHUNK]


def _emit_pipeline(nc, tc, n_chunks, low_chunks, k_blk, n_nodes, split,
                   handles, reps=1, n_queues=1):
    v_h, e_h, idx_h, w1_h, w2_h, out_h = handles
    chunk = P * k_blk
    idx_cols = chunk // 16

    with (
        tc.tile_pool(name="const", bufs=1) as const,
        tc.tile_pool(name="chunkio", bufs=4) as chunkio,
        tc.tile_pool(name="work", bufs=3) as work,
        tc.tile_pool(name="psum", bufs=8, space="PSUM") as psum,
    ):
        # --- constants -----------------------------------------------------
        w1t = const.tile([ATOM, MSG], BF16)       # W.T rows 0:128 (atom part)
        nc.sync.dma_start(out=w1t[:], in_=w1_h[:, :])
        w2tb = const.tile([BOND + 1, MSG], BF16)  # [W.T rows 128:192; bias]
        nc.sync.dma_start(out=w2tb[:], in_=w2_h[:, :])

        # whole-core gather indices, preloaded in a few big DMAs
        idx_all = const.tile([P, n_chunks * idx_cols], I16)
        pre = min(4, n_chunks)
        nc.sync.dma_start(
            out=idx_all[:, 0:pre * idx_cols], in_=idx_h[:, 0:pre * idx_cols]
        )
        if n_chunks > pre:
            nc.sync.dma_start(
                out=idx_all[:, pre * idx_cols:],
                in_=idx_h[:, pre * idx_cols:],
            )

        # --- main loop -----------------------------------------------------
        def chunk_body():
            for c in range(n_chunks):
                c0 = c * chunk
                c1 = c0 + chunk
                v_base = (
                    v_h[0:split, :] if c < low_chunks else v_h[split:n_nodes, :]
                )

                # feature-major gathered V: [128 atom features, chunk edges]
                v_t = chunkio.tile([P, chunk], BF16, tag="vgat")
                nc.gpsimd.dma_gather(
                    out_ap=v_t[:].rearrange("p (o k) -> p o k", o=1),
                    in_ap=v_base,
                    idxs_ap=idx_all[:, c * idx_cols:(c + 1) * idx_cols],
                    num_idxs=chunk,
                    num_idxs_reg=chunk,
                    elem_size=ATOM,
                    transpose=True,
                    single_packet=False,
                    queue_num=c % n_queues,
                )
                # feature-major E (+ ones row): [65, chunk edges]
                e_t = chunkio.tile([BOND + 1, chunk], BF16, tag="ebond")
                nc.scalar.dma_start(out=e_t[:], in_=e_h[:, c0:c1])

                o_grp = work.tile([P, k_blk * MSG], BF16, tag="ogrp")

                for j2 in range(0, k_blk, 2):
                    o_ps = psum.tile([P, 2 * MSG], F32, tag="ops")
                    for u in range(2):
                        j = j2 + u
                        nc.tensor.matmul(
                            out=o_ps[:, u * MSG:(u + 1) * MSG],
                            lhsT=v_t[:, j * P:(j + 1) * P],
                            rhs=w1t[:],
                            start=True,
                            stop=False,
                        )
                        nc.tensor.matmul(
                            out=o_ps[:, u * MSG:(u + 1) * MSG],
                            lhsT=e_t[:, j * P:(j + 1) * P],
                            rhs=w2tb[:],
                            start=False,
                            stop=True,
                        )
                    # PSUM -> SBUF evacuation with f32 -> bf16 cast; bias is
                    # already in PSUM via the ones-row x bias-row matmul.
                    dst = o_grp[:, j2 * MSG:(j2 + 2) * MSG]
                    if (j2 // 2) % 2 == 0:
                        nc.vector.tensor_copy(out=dst, in_=o_ps[:])
                    else:
                        nc.scalar.copy(out=dst, in_=o_ps[:])

                # slot s = c0 + p*k_blk + j holds position j*128+p: each
                # partition stores k_blk contiguous HBM rows (8KB descs)
                out_view = out_h[c0:c1, :].rearrange("(p k) m -> p k m", p=P)
                nc.sync.dma_start(out=out_view[:, :, :], in_=o_grp[:, :])

        if reps == 1:
            chunk_body()
        else:
            with tc.For_i(0, reps, 1):
                chunk_body()


def build_nc(n_chunks=N_CHUNKS, low_chunks=LOW_CHUNKS, k_blk=K_BLK,
             n_nodes=N_NODES, split=SPLIT, reps=1, n_queues=1):
    chunk = P * k_blk
    per_core = n_chunks * chunk
    idx_cols = chunk // 16

    nc = bacc.Bacc(num_swdge_queues=n_queues)
    handles = (
        nc.declare_dram_parameter("V", [n_nodes, ATOM], BF16, isOutput=False),
        nc.declare_dram_parameter("Et", [BOND + 1, per_core], BF16,
                                  isOutput=False),
        nc.declare_dram_parameter("idx16", [P, n_chunks * idx_cols], I16,
                                  isOutput=False),
        nc.declare_dram_parameter("W1t", [ATOM, MSG], BF16, isOutput=False),
        nc.declare_dram_parameter("W2tb", [BOND + 1, MSG], BF16,
                                  isOutput=False),
        nc.declare_dram_parameter("out", [per_core, MSG], BF16, isOutput=True),
    )
    with tile.TileContext(nc) as tc:
        _emit_pipeline(nc, tc, n_chunks, low_chunks, k_blk, n_nodes, split,
                       handles, reps=reps, n_queues=n_queues)
    return nc


def build_nc_null():
    """Null kernel with identical I/O signature — for RPC/transfer calibration."""
    nc = bacc.Bacc()
    nc.declare_dram_parameter("V", [N_NODES, ATOM], BF16, isOutput=False)
    nc.declare_dram_parameter("Et", [BOND + 1, PER_CORE], BF16, isOutput=False)
    nc.declare_dram_parameter("idx16", [P, N_CHUNKS * IDX_COLS], I16,
                              isOutput=False)
    w1_h = nc.declare_dram_parameter("W1t", [ATOM, MSG], BF16, isOutput=False)
    nc.declare_dram_parameter("W2tb", [BOND + 1, MSG], BF16, isOutput=False)
    out_h = nc.declare_dram_parameter("out", [PER_CORE, MSG], BF16,
                                      isOutput=True)
    with tile.TileContext(nc) as tc:
        with tc.tile_pool(name="p", bufs=1) as pool:
            t = pool.tile([P, MSG], BF16)
            nc.sync.dma_start(out=t[:], in_=w1_h[0:P, :])
            nc.sync.dma_start(out=out_h[0:P, :], in_=t[:])
    return nc


_NC_CACHE = {}


def _get_nc(key, **kw):
    if key not in _NC_CACHE:
        nc = (build_nc_null if key == "null" else build_nc)(**kw)
        nc.finalize()  # run Bacc passes (reg alloc, matmul wait legalization)
        _NC_CACHE[key] = nc
    return _NC_CACHE[key]


def wrap_idx16_chunks(idx_pos):
    """Wrap position-ordered gather indices for dma_gather.

    idx_pos: [n_chunks, chunk] int array, position i of chunk c gathers
    row idx_pos[c, i]. The ucode reads position i from
    [partition i % 16, col i // 16], replicated across the 8 16-partition
    bands; chunks are concatenated along the free dim.
    Returns [128, n_chunks * chunk // 16] int16.
    """
    n_chunks, chunk = idx_pos.shape
    cols = chunk // 16
    a = idx_pos.reshape(n_chunks, cols, 16).transpose(0, 2, 1)  # [nc,16,cols]
    a = np.tile(a, (1, 8, 1))                                   # [nc,128,cols]
    return np.ascontiguousarray(
        a.transpose(1, 0, 2).reshape(P, n_chunks * cols).astype(np.int16)
    )


def _make_in_maps(V, E, edge_index, W, b):
    V = np.asarray(V, dtype=np.float32)
    E = np.asarray(E, dtype=np.float32)
    W = np.asarray(W, dtype=np.float32)
    b = np.asarray(b, dtype=np.float32)

    src = np.asarray(edge_index[0]).astype(np.int32)
    v_bf = np.ascontiguousarray(V.astype(NP_BF16))
    wt = W.T.astype(NP_BF16)                       # [192, 256]
    w1t = np.ascontiguousarray(wt[:ATOM])          # [128, 256]
    w2tb = np.ascontiguousarray(
        np.concatenate([wt[ATOM:], b[None, :].astype(NP_BF16)], axis=0)
    )                                              # [65, 256]

    in_maps = []
    placements = []
    for i in range(N_CORES):
        lo = i * PER_CORE_RAW
        src_i = src[lo:lo + PER_CORE_RAW]
        e_i = E[lo:lo + PER_CORE_RAW]

        low_pos = np.flatnonzero(src_i < SPLIT)
        high_pos = np.flatnonzero(src_i >= SPLIT)
        n_low, n_high = len(low_pos), len(high_pos)
        assert n_low <= LOW_SLOTS and n_high <= HIGH_SLOTS, (n_low, n_high)

        # slot-ordered local edge ids (-1 = padding)
        slot_edge = np.full(PER_CORE, -1, np.int64)
        slot_edge[:n_low] = low_pos
        slot_edge[LOW_SLOTS:LOW_SLOTS + n_high] = high_pos

        # position-ordered view: position q of chunk c = slot POS2SLOT[q]
        pos_edge = slot_edge.reshape(N_CHUNKS, CHUNK)[:, POS2SLOT]

        # gather indices (pad -> 0)
        safe_edge = np.maximum(pos_edge, 0)
        idx_pos = src_i[safe_edge].astype(np.int32)
        idx_pos[N_CHUNKS // 2:] -= SPLIT           # high chunks use base SPLIT
        idx_pos[pos_edge < 0] = 0
        idx16 = wrap_idx16_chunks(idx_pos)

        # feature-major E with ones row, position-ordered columns
        e_pos = e_i[safe_edge.reshape(-1)].astype(NP_BF16)   # [PER_CORE, 64]
        e_pos[pos_edge.reshape(-1) < 0] = 0
        et = np.empty((BOND + 1, PER_CORE), NP_BF16)
        et[:BOND] = e_pos.T
        et[BOND] = 1.0

        in_maps.append(
            {
                "V": v_bf,
                "Et": np.ascontiguousarray(et),
                "idx16": idx16,
                "W1t": w1t,
                "W2tb": w2tb,
            }
        )
        placements.append(slot_edge)
    return in_maps, placements


def kernel(V, E, edge_index, W, b):
    in_maps, placements = _make_in_maps(V, E, edge_index, W, b)
    nc = _get_nc("full")
    res = run_bass_kernel_spmd(nc, in_maps, core_ids=list(range(N_CORES)))
    out = np.empty((N_EDGES, MSG), np.float32)
    for i, slot_edge in enumerate(placements):
        dev = np.asarray(res.results[i]["out"])
        valid = slot_edge >= 0
        blk = out[i * PER_CORE_RAW:(i + 1) * PER_CORE_RAW]
        blk[slot_edge[valid]] = dev[valid].astype(np.float32)
    return out


def kernel_null(V, E, edge_index, W, b):
    """Calibration: same transfers as kernel(), trivial device work."""
    in_maps, _ = _make_in_maps(V, E, edge_index, W, b)
    nc = _get_nc("null")
    res = run_bass_kernel_spmd(nc, in_maps, core_ids=list(range(N_CORES)))
    return res.results[0]["out"][0, 0]
